# revision 28
# baseline (speedup 1.0000x reference)
"""Trainium2 Bass kernel for a 2-layer GAT (PyG GATConv semantics, eval mode).

V2 strategy (8 NeuronCores, SPMD, dst-sharded):
  - Nodes sharded by destination: core c owns dst nodes [c*NPC, (c+1)*NPC).
  - Phase A: h1|a_src|a_dst = x @ W1aug per 128-node block (bf16).  [h1|a_src]
    rows (256B) go to a DRAM slab; a_dst rows go to a local DRAM table (adl)
    for per-edge gathers; a_src/a_dst/h also stay resident in SBUF for the
    analytic self-loop contribution.
  - The slab is AllGathered in 2 segments (overlapped with compute) into a
    shared 100352-row bf16 table; gathers use int16 indices over 4 chunks of
    25088 rows.
  - Edges (self-loops excluded) are bucketed by (dst block, chunk), with
    per-(block,chunk) tile counts taken from the actual data (max over cores
    so the SPMD program is identical).  Blocks are packed into groups of
    <= GT tiles; per (group, chunk) one dma_gather fetches [h|a_src] rows and
    one per-group gather fetches a_dst rows by local dst id (256B rows).
  - Group-wide vector ops compute w = exp(leaky_relu(a_src + a_dst)) and
    S = [w | w*h] in bf16; per 128-node dst block a 0/1 mask (built by one
    is_equal) aggregates S via PSUM-accumulated mask matmuls.
  - Epilogue per block adds the analytic self-loop term, divides by the
    softmax denominator, applies bias+ELU, and immediately computes layer-2
    rows h2|a_src2|a_dst2, writing slab2/adl2.  After segmented AllGathers
    of table2 (fired mid-phase for overlap), the same edge pipeline runs for
    layer 2 (single head), followed by log_softmax.
"""

import sys

if "/opt/trn_rl_repo" not in sys.path:
    sys.path.insert(0, "/opt/trn_rl_repo")

from dataclasses import dataclass

import numpy as np
import ml_dtypes

import concourse.bass as bass
import concourse.bacc as bacc
import concourse.tile as tile
import concourse.mybir as mybir
from concourse.masks import make_identity

F32 = mybir.dt.float32
BF16 = mybir.dt.bfloat16
I16 = mybir.dt.int16

NPBF = ml_dtypes.bfloat16
NEG_SLOPE = 0.2
PAD_DST = 584.0  # bf16-exact, > 127 so is_equal vs iota never matches


@dataclass(frozen=True)
class Cfg:
    N: int = 100000
    F: int = 128
    H1: int = 8
    C1: int = 8
    D2: int = 40
    NC: int = 8
    GT: int = 60  # max tiles per gather group
    dbg: bool = False

    @property
    def D1(self):
        return self.H1 * self.C1  # 64

    @property
    def NPC(self):
        return self.N // self.NC  # 12500

    @property
    def NB(self):
        return (self.NPC + 127) // 128  # 98

    @property
    def NPCP(self):
        return self.NB * 128  # 12544

    @property
    def SEGB(self):
        return (self.NB + 1) // 2  # 49 blocks per table segment

    @property
    def SEGR(self):
        return self.SEGB * 128  # 6272 local rows per segment

    @property
    def TROWS(self):
        return self.NC * self.NPCP  # 100352

    @property
    def CH(self):
        return self.TROWS // 4  # 25088 rows per gather chunk


# ---------------------------------------------------------------- host side


class Meta:
    """Static (core-independent) structure of the edge pipeline."""

    def __init__(self, T, cfg: Cfg):
        c = cfg
        self.T = T  # [NB][4] tiles per (block, chunk)
        self.TPB = [sum(T[b]) for b in range(c.NB)]
        # pack blocks into groups of <= GT tiles
        self.groups = []  # list of list of block ids
        cur, cur_t = [], 0
        for b in range(c.NB):
            if cur and cur_t + self.TPB[b] > c.GT:
                self.groups.append(cur)
                cur, cur_t = [], 0
            cur.append(b)
            cur_t += self.TPB[b]
        if cur:
            self.groups.append(cur)
        # per-group layout: chunk-major slot space
        self.ginfo = []
        base = 0  # global tile index
        for blocks in self.groups:
            calls = []  # (chunk, tile_off_in_group, ntiles)
            btiles = {b: [] for b in blocks}  # block -> list of global-ish tile idx
            off = 0
            for k in range(4):
                nt = sum(T[b][k] for b in blocks)
                if nt:
                    calls.append((k, off, nt))
                o = off
                for b in blocks:
                    for j in range(T[b][k]):
                        btiles[b].append(o + j)
                    o += T[b][k]
                off += nt
            self.ginfo.append(
                {"blocks": blocks, "calls": calls, "btiles": btiles,
                 "sl": off, "base": base}
            )
            base += off
        self.tot_tiles = base

    @property
    def nbmax(self):
        return max(len(g) for g in self.groups)

    @property
    def key(self):
        return tuple(tuple(t) for t in self.T)


def _wrap_idx(arr):
    """[L] int16 -> [128, L//16] wrapped in 16 partitions, replicated x8."""
    L = arr.shape[0]
    w = arr.reshape(L // 16, 16).T  # [16, L//16]
    return np.tile(w, (8, 1)).astype(np.int16)  # [128, L//16]


def preprocess(x, edge_index, cfg: Cfg):
    c = cfg
    src = np.asarray(edge_index[0]).astype(np.int64)
    dst = np.asarray(edge_index[1]).astype(np.int64)

    loc = src % c.NPC
    seg = (loc >= c.SEGR).astype(np.int64)
    trow_all = seg * (c.NC * c.SEGR) + (src // c.NPC) * c.SEGR + (loc - seg * c.SEGR)

    per_core = []
    counts = np.zeros((c.NC, c.NB, 4), np.int64)
    for core in range(c.NC):
        lo, hi = core * c.NPC, (core + 1) * c.NPC
        m = (dst >= lo) & (dst < hi)
        t_r, d_l = trow_all[m], dst[m] - lo
        blk = d_l >> 7
        chunk = t_r // c.CH
        order = np.lexsort((t_r, chunk, blk))
        t_r, d_l, blk, chunk = t_r[order], d_l[order], blk[order], chunk[order]
        key = blk * 4 + chunk
        uniq, cnt = np.unique(key, return_counts=True)
        counts[core].reshape(-1)[uniq] = cnt
        per_core.append((t_r, d_l, key))

    cmax = counts.max(axis=0)  # [NB, 4]
    T = [[int(-(-cmax[b, k] // 128)) for k in range(4)] for b in range(c.NB)]
    meta = Meta(T, cfg)

    in_maps = []
    iota_row = np.tile(np.arange(128, dtype=np.float32), (128, 1)).astype(NPBF)
    iotac = np.arange(128, dtype=np.float32).reshape(128, 1).astype(NPBF)
    ones_row = np.ones((1, 128), np.float32)
    blockmask = np.zeros((c.D1, c.H1), np.float32)
    for h in range(c.H1):
        blockmask[h * c.C1 : (h + 1) * c.C1, h] = 1.0

    TT = meta.tot_tiles
    for core in range(c.NC):
        t_r, d_l, key = per_core[core]
        # run boundaries per (b, k) in the sorted arrays
        starts = np.searchsorted(key, np.arange(c.NB * 4))
        ends = np.searchsorted(key, np.arange(c.NB * 4) + 1)

        dstib = np.full((128, TT), PAD_DST, np.float32)
        dstib2 = np.full((1, TT * 128), PAD_DST, np.float32)
        gidx_arr = np.zeros((128, TT * 8), np.int16)

        for gi, g in enumerate(meta.ginfo):
            base = g["base"]
            # per (chunk, block) segments in chunk-major group order
            for (k, off, nt) in g["calls"]:
                ids = []
                o = off
                for b in g["blocks"]:
                    Tbk = meta.T[b][k]
                    if Tbk == 0:
                        continue
                    s, e = starts[b * 4 + k], ends[b * 4 + k]
                    n = e - s
                    idx = np.zeros(Tbk * 128, np.int16)
                    idx[:n] = (t_r[s:e] - k * c.CH).astype(np.int16)
                    ids.append(idx)
                    # dstib for these slots
                    i = np.arange(n)
                    vals = (d_l[s:e] - (b << 7)).astype(np.float32)
                    dstib[i % 128, base + o + i // 128] = vals
                    dstib2[0, (base + o) * 128 + i] = vals
                    o += Tbk
                ids = np.concatenate(ids)
                gidx_arr[:, (base + off) * 8 : (base + off) * 8 + nt * 8] = _wrap_idx(
                    ids
                )

        lo = core * c.NPC
        xsT = np.zeros((c.F, c.NPCP), np.float32)
        xsT[:, : c.NPC] = np.asarray(x)[lo : lo + c.NPC].T

        in_maps.append(
            {
                "xT": xsT,
                "dstib": dstib.astype(NPBF),
                "dstib2": dstib2.astype(NPBF),
                "gidx": gidx_arr,
                "iota_row": iota_row,
                "iotac": iotac,
                "ones_row": ones_row,
                "blockmask": blockmask,
            }
        )
    return in_maps, meta


# -------------------------------------------------------------- device side


def build(nc, cfg: Cfg, meta: Meta, repeats: int = 1):
    c = cfg
    D1, D2, H1 = c.D1, c.D2, c.H1

    xT_t = nc.dram_tensor("xT", [c.F, c.NPCP], F32, kind="ExternalInput")
    W1 = nc.dram_tensor("W1", [c.F, D1], F32, kind="ExternalInput")
    att_src1 = nc.dram_tensor("att_src1", [H1, c.C1], F32, kind="ExternalInput")
    att_dst1 = nc.dram_tensor("att_dst1", [H1, c.C1], F32, kind="ExternalInput")
    b1 = nc.dram_tensor("b1", [D1], F32, kind="ExternalInput")
    W2 = nc.dram_tensor("W2", [D1, D2], F32, kind="ExternalInput")
    att_src2 = nc.dram_tensor("att_src2", [1, D2], F32, kind="ExternalInput")
    att_dst2 = nc.dram_tensor("att_dst2", [1, D2], F32, kind="ExternalInput")
    b2 = nc.dram_tensor("b2", [D2], F32, kind="ExternalInput")
    dstib_t = nc.dram_tensor("dstib", [128, meta.tot_tiles], BF16, kind="ExternalInput")
    dstib2_t = nc.dram_tensor(
        "dstib2", [1, meta.tot_tiles * 128], BF16, kind="ExternalInput"
    )
    gidx_t = nc.dram_tensor("gidx", [128, meta.tot_tiles * 8], I16, kind="ExternalInput")
    iota_t = nc.dram_tensor("iota_row", [128, 128], BF16, kind="ExternalInput")
    iotac_t = nc.dram_tensor("iotac", [128, 1], BF16, kind="ExternalInput")
    ones_t = nc.dram_tensor("ones_row", [1, 128], F32, kind="ExternalInput")
    bmask_t = nc.dram_tensor("blockmask", [D1, H1], F32, kind="ExternalInput")
    out_t = nc.dram_tensor("out", [c.NPCP, D2], F32, kind="ExternalOutput")

    SEGR, SEGB = c.SEGR, c.SEGB
    TSEG = c.NC * SEGR  # 50176 table rows per segment

    with tile.TileContext(nc) as tc:
        with (
            tc.tile_pool(name="dram", bufs=1, space="DRAM") as dram,
            tc.tile_pool(name="const", bufs=1) as cst,
            tc.tile_pool(name="pers", bufs=1) as pers,
            tc.tile_pool(name="work", bufs=2) as wk,
            tc.tile_pool(name="gath", bufs=2) as gp,
            tc.tile_pool(name="mask", bufs=2) as mk,
            tc.tile_pool(name="maskT", bufs=2) as mkT,
            tc.tile_pool(name="drep", bufs=2) as dpp,
            tc.tile_pool(name="sbuf_s", bufs=2) as sp,
            tc.tile_pool(name="psum", bufs=1, space="PSUM") as ps,
            tc.tile_pool(name="psumT", bufs=1, space="PSUM") as psT,
            tc.tile_pool(name="psdr", bufs=1, space="PSUM") as drp_pool,
            tc.tile_pool(name="psea", bufs=1, space="PSUM") as eap,
            tc.tile_pool(name="psagg", bufs=1, space="PSUM") as agp,
        ):
            # ---- constants
            ident = cst.tile([128, 128], F32)
            make_identity(nc, ident[:])
            identb = cst.tile([128, 128], BF16)
            nc.vector.tensor_copy(out=identb[:], in_=ident[:])
            iota = cst.tile([128, 128], BF16)
            nc.sync.dma_start(out=iota[:], in_=iota_t.ap())
            iotac = cst.tile([128, 1], BF16)
            nc.sync.dma_start(out=iotac[:], in_=iotac_t.ap())
            iotac_rep = cst.tile([128, 1024], BF16)
            nc.vector.tensor_copy(
                out=iotac_rep[:], in_=iotac[:].to_broadcast([128, 1024])
            )
            ones1 = cst.tile([1, 128], F32)
            nc.sync.dma_start(out=ones1[:], in_=ones_t.ap())
            ones1b = cst.tile([1, 128], BF16)
            nc.vector.tensor_copy(out=ones1b[:], in_=ones1[:])

            w1sb = cst.tile([c.F, D1], F32)
            nc.sync.dma_start(out=w1sb[:], in_=W1.ap())
            w1T_ps = psT.tile([D1, c.F], F32, tag="pT")
            nc.tensor.transpose(out=w1T_ps[:], in_=w1sb[:], identity=ident[:])
            w1T = cst.tile([D1, c.F], F32)
            nc.vector.tensor_copy(out=w1T[:], in_=w1T_ps[:])

            bmask = cst.tile([D1, H1], F32)
            nc.sync.dma_start(out=bmask[:], in_=bmask_t.ap())
            atts_c = cst.tile([D1, 2], F32)
            nc.sync.dma_start(
                out=atts_c[:, 0:1], in_=att_src1.ap().rearrange("h c -> (h c)")[:, None]
            )
            nc.sync.dma_start(
                out=atts_c[:, 1:2], in_=att_dst1.ap().rearrange("h c -> (h c)")[:, None]
            )
            ablk = cst.tile([D1, 2 * H1], F32)
            nc.vector.tensor_tensor(
                out=ablk[:, 0:H1],
                in0=atts_c[:, 0:1].to_broadcast([D1, H1]),
                in1=bmask[:],
                op=mybir.AluOpType.mult,
            )
            nc.vector.tensor_tensor(
                out=ablk[:, H1 : 2 * H1],
                in0=atts_c[:, 1:2].to_broadcast([D1, H1]),
                in1=bmask[:],
                op=mybir.AluOpType.mult,
            )
            acols_ps = psT.tile([c.F, 2 * H1], F32, tag="pT")
            nc.tensor.matmul(
                out=acols_ps[:], lhsT=w1T[:], rhs=ablk[:], start=True, stop=True
            )
            w1aug = cst.tile([c.F, D1 + 2 * H1], BF16)
            nc.vector.tensor_copy(out=w1aug[:, 0:D1], in_=w1sb[:])
            nc.vector.tensor_copy(out=w1aug[:, D1 : D1 + 2 * H1], in_=acols_ps[:])

            w2sb = cst.tile([D1, D2], F32)
            nc.sync.dma_start(out=w2sb[:], in_=W2.ap())
            w2T_ps = psT.tile([D2, D1], F32, tag="pT")
            nc.tensor.transpose(out=w2T_ps[:], in_=w2sb[:], identity=ident[0:D1, 0:D1])
            w2T = cst.tile([D2, D1], F32)
            nc.vector.tensor_copy(out=w2T[:], in_=w2T_ps[:])
            att2 = cst.tile([D2, 2], F32)
            nc.sync.dma_start(
                out=att2[:, 0:1], in_=att_src2.ap().rearrange("o c -> (o c)")[:, None]
            )
            nc.sync.dma_start(
                out=att2[:, 1:2], in_=att_dst2.ap().rearrange("o c -> (o c)")[:, None]
            )
            v2_ps = psT.tile([D1, 2], F32, tag="pT")
            nc.tensor.matmul(out=v2_ps[:], lhsT=w2T[:], rhs=att2[:], start=True, stop=True)
            w2aug = cst.tile([D1, D2 + 2], BF16)
            nc.vector.tensor_copy(out=w2aug[:, 0:D2], in_=w2sb[:])
            nc.vector.tensor_copy(out=w2aug[:, D2 : D2 + 2], in_=v2_ps[:])

            b1row = cst.tile([1, D1], F32)
            nc.sync.dma_start(out=b1row[:], in_=b1.ap()[None, :])
            b1rep_ps = psT.tile([128, D1], F32, tag="pT")
            nc.tensor.matmul(
                out=b1rep_ps[:], lhsT=ones1[:], rhs=b1row[:], start=True, stop=True
            )
            b1rep = cst.tile([128, D1], F32)
            nc.vector.tensor_copy(out=b1rep[:], in_=b1rep_ps[:])
            b2row = cst.tile([1, D2], F32)
            nc.sync.dma_start(out=b2row[:], in_=b2.ap()[None, :])
            b2rep_ps = psT.tile([128, D2], F32, tag="pT")
            nc.tensor.matmul(
                out=b2rep_ps[:], lhsT=ones1[:], rhs=b2row[:], start=True, stop=True
            )
            b2rep = cst.tile([128, D2], F32)
            nc.vector.tensor_copy(out=b2rep[:], in_=b2rep_ps[:])
            NBM = meta.nbmax

            # persistent per-node data (bf16)
            asrc_all = pers.tile([128, c.NB * H1], BF16)
            adst_all = pers.tile([128, c.NB * H1], BF16)
            h_all = pers.tile([128, c.NB * D1], BF16)
            h2s_all = pers.tile([128, c.NB * (D2 + 2)], BF16)
            # per-layer batched self-loop terms, interleaved [esl | esl*h] per
            # block so one identity-matmul per block accumulates them into the
            # PSUM aggregate
            eslwhl = pers.tile([128, c.NB * 72], BF16)

            # tiny warm-up collective: absorbs the CC rendezvous/ring-setup
            # cost (~120us) so the first real AllGather starts promptly
            warm_in = dram.tile([16, 128], BF16, tag="warm_in", name="warm_in")
            warm_out = dram.tile(
                [128, 128], BF16, addr_space="Shared", tag="warm_out", name="warm_out"
            )
            nc.sync.dma_start(out=warm_in[:], in_=iota_t.ap()[0:16, :])
            nc.gpsimd.collective_compute(
                "AllGather",
                mybir.AluOpType.bypass,
                replica_groups=[list(range(c.NC))],
                ins=[warm_in.opt()],
                outs=[warm_out.opt()],
            )

            for _rep in range(repeats):
                slab1 = [
                    dram.tile([SEGR, 128], BF16, tag=f"s1{s}_{_rep}", name=f"s1{s}_{_rep}") for s in range(2)
                ]
                table1 = [
                    dram.tile([TSEG, 128], BF16, addr_space="Shared", tag=f"t1{s}_{_rep}", name=f"t1{s}_{_rep}")
                    for s in range(2)
                ]
                slab2 = [
                    dram.tile([SEGR, 128], BF16, tag=f"s2{s}_{_rep}", name=f"s2{s}_{_rep}") for s in range(2)
                ]
                table2 = [
                    dram.tile([TSEG, 128], BF16, addr_space="Shared", tag=f"t2{s}_{_rep}", name=f"t2{s}_{_rep}")
                    for s in range(2)
                ]

                # ---- phase A
                for t in range(c.NB):
                    xt = wk.tile([128, 128], F32, tag="xt")
                    nc.sync.dma_start(
                        out=xt[:], in_=xT_t.ap()[:, t * 128 : (t + 1) * 128]
                    )
                    xtb = wk.tile([128, 128], BF16, tag="xtb")
                    nc.scalar.activation(
                        out=xtb[:], in_=xt[:],
                        func=mybir.ActivationFunctionType.Copy,
                    )
                    h_ps = ps.tile([128, D1 + 2 * H1], F32, tag="hps", bufs=2)
                    nc.tensor.matmul(
                        out=h_ps[:], lhsT=xtb[:], rhs=w1aug[:], start=True, stop=True
                    )
                    s1o = wk.tile([128, 128], BF16, tag="s1o")
                    nc.scalar.activation(
                        out=s1o[:, 0 : D1 + H1], in_=h_ps[:, 0 : D1 + H1],
                        func=mybir.ActivationFunctionType.Copy,
                    )
                    seg, r = divmod(t, SEGB)
                    nc.sync.dma_start(
                        out=slab1[seg][r * 128 : (r + 1) * 128, :], in_=s1o[:]
                    )
                    nc.vector.tensor_copy(
                        out=asrc_all[:, t * H1 : (t + 1) * H1],
                        in_=h_ps[:, D1 : D1 + H1],
                    )
                    nc.vector.tensor_copy(
                        out=adst_all[:, t * H1 : (t + 1) * H1],
                        in_=h_ps[:, D1 + H1 : D1 + 2 * H1],
                    )
                    nc.vector.tensor_copy(
                        out=h_all[:, t * D1 : (t + 1) * D1], in_=h_ps[:, 0:D1]
                    )
                    if t == SEGB - 1 or t == c.NB - 1:
                        seg = 0 if t == SEGB - 1 else 1
                        nc.gpsimd.collective_compute(
                            "AllGather",
                            mybir.AluOpType.bypass,
                            replica_groups=[list(range(c.NC))],
                            ins=[slab1[seg].opt()],
                            outs=[table1[seg].opt()],
                        )

                # ---- edge pipeline (shared between layers)
                def edge_layer(layer):
                    NH = H1 if layer == 1 else 1
                    DV = D1 if layer == 1 else D2
                    SW = NH + DV
                    table = table1 if layer == 1 else table2
                    # batched self-loop precompute over all blocks, written into
                    # the interleaved [esl | esl*h] layout matching agg PSUM
                    ew3 = eslwhl[:, 0 : c.NB * SW].rearrange(
                        "p (bl s) -> p bl s", s=SW
                    )
                    if layer == 1:
                        nc.vector.tensor_tensor(
                            out=ew3[:, :, 0:NH],
                            in0=asrc_all[:].rearrange("p (bl h) -> p bl h", h=NH),
                            in1=adst_all[:].rearrange("p (bl h) -> p bl h", h=NH),
                            op=mybir.AluOpType.add,
                        )
                    else:
                        h2s4 = h2s_all[:].rearrange("p (bl d) -> p bl d", d=42)
                        nc.vector.tensor_tensor(
                            out=ew3[:, :, 0:NH],
                            in0=h2s4[:, :, 40:41],
                            in1=h2s4[:, :, 41:42],
                            op=mybir.AluOpType.add,
                        )
                    esl2 = wk.tile([128, c.NB * H1], BF16, tag="esl2")
                    nc.vector.tensor_scalar_mul(
                        out=esl2[:, 0 : c.NB * NH].rearrange(
                            "p (bl h) -> p bl h", h=NH
                        ),
                        in0=ew3[:, :, 0:NH],
                        scalar1=NEG_SLOPE,
                    )
                    nc.vector.tensor_tensor(
                        out=ew3[:, :, 0:NH],
                        in0=ew3[:, :, 0:NH],
                        in1=esl2[:, 0 : c.NB * NH].rearrange(
                            "p (bl h) -> p bl h", h=NH
                        ),
                        op=mybir.AluOpType.max,
                    )
                    nc.scalar.activation(
                        out=ew3[:, :, 0:NH],
                        in_=ew3[:, :, 0:NH],
                        func=mybir.ActivationFunctionType.Exp,
                    )
                    if layer == 1:
                        nc.vector.tensor_tensor(
                            out=ew3[:, :, NH:SW].rearrange(
                                "p bl (h ch) -> p bl h ch", h=NH
                            ),
                            in0=h_all[:].rearrange(
                                "p (bl h ch) -> p bl h ch", bl=c.NB, h=NH
                            ),
                            in1=ew3[:, :, 0:NH].to_broadcast(
                                [128, c.NB, NH, c.C1]
                            ),
                            op=mybir.AluOpType.mult,
                        )
                    else:
                        nc.vector.tensor_tensor(
                            out=ew3[:, :, NH:SW],
                            in0=h2s4[:, :, 0:D2],
                            in1=ew3[:, :, 0:NH].to_broadcast([128, c.NB, D2]),
                            op=mybir.AluOpType.mult,
                        )
                    for g in meta.ginfo:
                        SL, base = g["sl"], g["base"]
                        blocks = g["blocks"]
                        b0, nb = blocks[0], len(blocks)
                        dstib = wk.tile([128, c.GT], BF16, tag="dstib")
                        nc.sync.dma_start(
                            out=dstib[:, 0:SL], in_=dstib_t.ap()[:, base : base + SL]
                        )
                        d2 = wk.tile([1, c.GT * 128], BF16, tag="d2")
                        nc.sync.dma_start(
                            out=d2[:, 0 : SL * 128],
                            in_=dstib2_t.ap()[:, base * 128 : (base + SL) * 128],
                        )
                        gix = wk.tile([128, c.GT * 8], I16, tag="gix")
                        nc.sync.dma_start(
                            out=gix[:, 0 : SL * 8],
                            in_=gidx_t.ap()[:, base * 8 : (base + SL) * 8],
                        )
                        gg = gp.tile([128, c.GT * 128], BF16, tag="gg")
                        gg3 = gg[:].rearrange("p (t e) -> p t e", e=128)
                        for (k, off, nt) in g["calls"]:
                            nc.gpsimd.dma_gather(
                                out_ap=gg3[:, off : off + nt, :],
                                in_ap=table[k // 2][:][
                                    (k % 2) * c.CH : (k % 2 + 1) * c.CH, :
                                ],
                                idxs_ap=gix[:, off * 8 : (off + nt) * 8],
                                num_idxs=nt * 128,
                                num_idxs_reg=nt * 128,
                                elem_size=128,
                                single_packet=False,
                                queue_num=k,
                            )
                        # replicate dstib2 across partitions via K=1 matmuls,
                        # then build maskT chunk by chunk
                        maskT = mkT.tile([128, c.GT * 128], BF16, tag="maskT")
                        CHK = 1024
                        for lo in range(0, SL * 128, CHK):
                            hi = min(SL * 128, lo + CHK)
                            drp = drp_pool.tile([128, CHK], F32, tag="drp")
                            for mo in range(lo, hi, 512):
                                me = min(hi, mo + 512)
                                nc.tensor.matmul(
                                    out=drp[:, mo - lo : me - lo],
                                    lhsT=ones1b[:],
                                    rhs=d2[:, mo:me],
                                    start=True,
                                    stop=True,
                                )
                            d2rep = dpp.tile([128, CHK], BF16, tag="d2rep")
                            nc.scalar.activation(
                                out=d2rep[:, 0 : hi - lo],
                                in_=drp[:, 0 : hi - lo],
                                func=mybir.ActivationFunctionType.Copy,
                            )
                            nc.vector.tensor_tensor(
                                out=maskT[:, lo:hi],
                                in0=d2rep[:, 0 : hi - lo],
                                in1=iotac_rep[:, 0 : hi - lo],
                                op=mybir.AluOpType.is_equal,
                            )
                        mask = mk.tile([128, c.GT * 128], BF16, tag="mask")
                        nc.vector.tensor_tensor(
                            out=mask[:, 0 : SL * 128],
                            in0=dstib[:, 0:SL].to_broadcast([128, SL, 128]),
                            in1=iota[:][:, None, :].to_broadcast([128, SL, 128]),
                            op=mybir.AluOpType.is_equal,
                        )
                        # per-slot a_dst via maskT matmuls: ea[e, NH] per tile
                        ea = eap.tile([128, c.GT * H1], F32, tag="ea")
                        for b in blocks:
                            if layer == 1:
                                adst_b = adst_all[:, b * H1 : (b + 1) * H1]
                            else:
                                adst_b = h2s_all[:, b * 42 + 41 : b * 42 + 42]
                            for jt in g["btiles"][b]:
                                nc.tensor.matmul(
                                    out=ea[:, jt * NH : (jt + 1) * NH],
                                    lhsT=maskT[:, jt * 128 : (jt + 1) * 128],
                                    rhs=adst_b,
                                    start=True,
                                    stop=True,
                                )
                        eab = wk.tile([128, c.GT * NH], BF16, tag="eab")
                        nc.scalar.activation(
                            out=eab[:, 0 : SL * NH],
                            in_=ea[:, 0 : SL * NH],
                            func=mybir.ActivationFunctionType.Copy,
                        )
                        # w = exp(lrelu(a_src + a_dst))  (group-wide)
                        et = wk.tile([128, c.GT * NH], BF16, tag="et")
                        e3 = et[:].rearrange("p (t h) -> p t h", h=NH)
                        nc.vector.tensor_tensor(
                            out=e3[:, 0:SL, :],
                            in0=gg3[:, 0:SL, DV : DV + NH],
                            in1=eab[:].rearrange("p (t h) -> p t h", h=NH)[:, 0:SL, :],
                            op=mybir.AluOpType.add,
                        )
                        et2 = wk.tile([128, c.GT * NH], BF16, tag="et2")
                        nc.vector.tensor_scalar_mul(
                            out=et2[:, 0 : SL * NH],
                            in0=et[:, 0 : SL * NH],
                            scalar1=NEG_SLOPE,
                        )
                        nc.vector.tensor_tensor(
                            out=et[:, 0 : SL * NH],
                            in0=et[:, 0 : SL * NH],
                            in1=et2[:, 0 : SL * NH],
                            op=mybir.AluOpType.max,
                        )
                        S = sp.tile([128, c.GT * 72], BF16, tag="S")
                        S3 = S[:].rearrange("p (t e) -> p t e", e=72)
                        nc.scalar.activation(
                            out=S3[:, 0:SL, 0:NH],
                            in_=e3[:, 0:SL, :],
                            func=mybir.ActivationFunctionType.Exp,
                        )
                        if layer == 1:
                            nc.vector.tensor_tensor(
                                out=S3[:, 0:SL, NH:SW].rearrange(
                                    "p t (h ch) -> p t h ch", h=NH
                                ),
                                in0=gg3[:, 0:SL, 0:DV].rearrange(
                                    "p t (h ch) -> p t h ch", h=NH
                                ),
                                in1=S3[:, 0:SL, 0:NH].to_broadcast(
                                    [128, SL, NH, c.C1]
                                ),
                                op=mybir.AluOpType.mult,
                            )
                        else:
                            nc.vector.tensor_tensor(
                                out=S3[:, 0:SL, NH:SW],
                                in0=gg3[:, 0:SL, 0:DV],
                                in1=S3[:, 0:SL, 0:NH].to_broadcast([128, SL, DV]),
                                op=mybir.AluOpType.mult,
                            )
                        # aggregation: one PSUM region per block in the group
                        NBM = meta.nbmax
                        agg = agp.tile([128, NBM * (H1 + D1)], F32, tag="agg")
                        ag3 = agg[:, 0 : nb * SW].rearrange("p (bl s) -> p bl s", s=SW)
                        for bi, b in enumerate(blocks):
                            tiles = g["btiles"][b]
                            for i, jt in enumerate(tiles):
                                nc.tensor.matmul(
                                    out=agg[:, bi * SW : (bi + 1) * SW],
                                    lhsT=mask[:, jt * 128 : (jt + 1) * 128],
                                    rhs=S3[:, jt, 0:SW],
                                    start=(i == 0),
                                    stop=False,
                                )
                            # self-loop terms folded in via identity matmul
                            nc.tensor.matmul(
                                out=agg[:, bi * SW : (bi + 1) * SW],
                                lhsT=identb[:],
                                rhs=eslwhl[:, b * SW : (b + 1) * SW],
                                start=False,
                                stop=True,
                            )
                        rec = wk.tile([128, NBM * NH], F32, tag="rec")
                        nc.vector.reciprocal(
                            out=rec[:, 0 : nb * NH].rearrange(
                                "p (bl h) -> p bl h", h=NH
                            ),
                            in_=ag3[:, 0:nb, 0:NH],
                        )
                        o1 = wk.tile([128, NBM * DV], F32, tag="o1")
                        if layer == 1:
                            nc.vector.tensor_tensor(
                                out=o1[:, 0 : nb * DV].rearrange(
                                    "p (bl h ch) -> p bl h ch", bl=nb, h=NH
                                ),
                                in0=ag3[:, 0:nb, NH:SW].rearrange(
                                    "p bl (h ch) -> p bl h ch", h=NH
                                ),
                                in1=rec[:, 0 : nb * NH]
                                .rearrange("p (bl h) -> p bl h", h=NH)
                                .to_broadcast([128, nb, NH, c.C1]),
                                op=mybir.AluOpType.mult,
                            )
                            nc.vector.tensor_tensor(
                                out=o1[:, 0 : nb * DV].rearrange(
                                    "p (bl d) -> p bl d", d=DV
                                ),
                                in0=o1[:, 0 : nb * DV].rearrange(
                                    "p (bl d) -> p bl d", d=DV
                                ),
                                in1=b1rep[:][:, None, :].to_broadcast(
                                    [128, nb, DV]
                                ),
                                op=mybir.AluOpType.add,
                            )
                            # ELU (batched): elu = max(x,0) + exp(min(x,0)) - 1
                            neg = wk.tile([128, NBM * DV], F32, tag="neg")
                            nc.vector.tensor_scalar_min(
                                out=neg[:, 0 : nb * DV], in0=o1[:, 0 : nb * DV],
                                scalar1=0.0,
                            )
                            nc.scalar.activation(
                                out=neg[:, 0 : nb * DV], in_=neg[:, 0 : nb * DV],
                                func=mybir.ActivationFunctionType.Exp,
                            )
                            nc.vector.tensor_scalar_max(
                                out=o1[:, 0 : nb * DV], in0=o1[:, 0 : nb * DV],
                                scalar1=0.0,
                            )
                            elu = neg
                            nc.vector.tensor_tensor(
                                out=elu[:, 0 : nb * DV], in0=o1[:, 0 : nb * DV],
                                in1=neg[:, 0 : nb * DV], op=mybir.AluOpType.add,
                            )
                            nc.vector.tensor_scalar_add(
                                out=elu[:, 0 : nb * DV], in0=elu[:, 0 : nb * DV],
                                scalar1=-1.0,
                            )
                            for bi, b in enumerate(blocks):
                                eT_ps = psT.tile([D1, 128], F32, tag="pT")
                                nc.tensor.transpose(
                                    out=eT_ps[:],
                                    in_=elu[:, bi * DV : (bi + 1) * DV],
                                    identity=ident[:],
                                )
                                eT = wk.tile([D1, 128], BF16, tag="eT")
                                nc.scalar.activation(
                                    out=eT[:], in_=eT_ps[:],
                                    func=mybir.ActivationFunctionType.Copy,
                                )
                                h2_ps = ps.tile([128, D2 + 2], F32, tag="h2ps")
                                nc.tensor.matmul(
                                    out=h2_ps[:], lhsT=eT[:], rhs=w2aug[:],
                                    start=True, stop=True,
                                )
                                s2o = wk.tile([128, 128], BF16, tag="s2o")
                                nc.scalar.activation(
                                    out=s2o[:, 0 : D2 + 1],
                                    in_=h2_ps[:, 0 : D2 + 1],
                                    func=mybir.ActivationFunctionType.Copy,
                                )
                                seg, r = divmod(b, SEGB)
                                nc.sync.dma_start(
                                    out=slab2[seg][r * 128 : (r + 1) * 128, :],
                                    in_=s2o[:],
                                )
                                nc.scalar.activation(
                                    out=h2s_all[:, b * 42 : b * 42 + 42],
                                    in_=h2_ps[:],
                                    func=mybir.ActivationFunctionType.Copy,
                                )
                                if b == SEGB - 1 or b == c.NB - 1:
                                    seg = 0 if b == SEGB - 1 else 1
                                    nc.gpsimd.collective_compute(
                                        "AllGather",
                                        mybir.AluOpType.bypass,
                                        replica_groups=[list(range(c.NC))],
                                        ins=[slab2[seg].opt()],
                                        outs=[table2[seg].opt()],
                                    )
                        else:
                            nc.vector.tensor_tensor(
                                out=o1[:, 0 : nb * DV].rearrange(
                                    "p (bl d) -> p bl d", d=DV
                                ),
                                in0=ag3[:, 0:nb, NH:SW],
                                in1=rec[:, 0:nb].to_broadcast([128, nb, DV]),
                                op=mybir.AluOpType.mult,
                            )
                            nc.vector.tensor_tensor(
                                out=o1[:, 0 : nb * DV].rearrange(
                                    "p (bl d) -> p bl d", d=DV
                                ),
                                in0=o1[:, 0 : nb * DV].rearrange(
                                    "p (bl d) -> p bl d", d=DV
                                ),
                                in1=b2rep[:, 0:D2][:, None, :].to_broadcast(
                                    [128, nb, DV]
                                ),
                                op=mybir.AluOpType.add,
                            )
                            # batched log_softmax
                            mx = wk.tile([128, NBM], F32, tag="mx")
                            nc.vector.tensor_reduce(
                                out=mx[:, 0:nb].rearrange("p (bl o) -> p bl o", o=1),
                                in_=o1[:, 0 : nb * DV].rearrange(
                                    "p (bl d) -> p bl d", d=DV
                                ),
                                axis=mybir.AxisListType.X,
                                op=mybir.AluOpType.max,
                            )
                            xm = wk.tile([128, NBM * DV], F32, tag="xm")
                            nc.vector.tensor_tensor(
                                out=xm[:, 0 : nb * DV].rearrange(
                                    "p (bl d) -> p bl d", d=DV
                                ),
                                in0=o1[:, 0 : nb * DV].rearrange(
                                    "p (bl d) -> p bl d", d=DV
                                ),
                                in1=mx[:, 0:nb].to_broadcast([128, nb, DV]),
                                op=mybir.AluOpType.subtract,
                            )
                            ex = wk.tile([128, NBM * DV], F32, tag="ex")
                            nc.scalar.activation(
                                out=ex[:, 0 : nb * DV], in_=xm[:, 0 : nb * DV],
                                func=mybir.ActivationFunctionType.Exp,
                            )
                            sm = wk.tile([128, NBM], F32, tag="sm")
                            nc.vector.tensor_reduce(
                                out=sm[:, 0:nb].rearrange("p (bl o) -> p bl o", o=1),
                                in_=ex[:, 0 : nb * DV].rearrange(
                                    "p (bl d) -> p bl d", d=DV
                                ),
                                axis=mybir.AxisListType.X,
                                op=mybir.AluOpType.add,
                            )
                            lg = wk.tile([128, NBM], F32, tag="lg")
                            nc.scalar.activation(
                                out=lg[:, 0:nb], in_=sm[:, 0:nb],
                                func=mybir.ActivationFunctionType.Ln,
                            )
                            oo = xm
                            nc.vector.tensor_tensor(
                                out=oo[:, 0 : nb * DV].rearrange(
                                    "p (bl d) -> p bl d", d=DV
                                ),
                                in0=xm[:, 0 : nb * DV].rearrange(
                                    "p (bl d) -> p bl d", d=DV
                                ),
                                in1=lg[:, 0:nb].to_broadcast([128, nb, DV]),
                                op=mybir.AluOpType.subtract,
                            )
                            for bi, b in enumerate(blocks):
                                nc.sync.dma_start(
                                    out=out_t.ap()[b * 128 : (b + 1) * 128, :],
                                    in_=oo[:, bi * DV : (bi + 1) * DV],
                                )

                edge_layer(1)
                edge_layer(2)


# ------------------------------------------------------------------ driver


def make_runner(nc, n_cores=8):
    """Build-once PJRT runner (reusable so repeated calls don't recompile)."""
    import jax
    from jax.sharding import Mesh, PartitionSpec
    from jax.experimental.shard_map import shard_map
    from concourse.bass2jax import (
        _bass_exec_p,
        install_neuronx_cc_hook,
        partition_id_tensor,
    )

    install_neuronx_cc_hook()
    partition_name = nc.partition_id_tensor.name if nc.partition_id_tensor else None

    in_names, out_names, out_avals, zero_outs = [], [], [], []
    for alloc in nc.m.functions[0].allocations:
        if not isinstance(alloc, mybir.MemoryLocationSet):
            continue
        name = alloc.memorylocations[0].name
        if alloc.kind == "ExternalInput":
            if name != partition_name:
                in_names.append(name)
        elif alloc.kind == "ExternalOutput":
            shape = tuple(alloc.tensor_shape)
            dtype = mybir.dt.np(alloc.dtype)
            out_names.append(name)
            out_avals.append(jax.core.ShapedArray(shape, dtype))
            zero_outs.append(np.zeros(shape, dtype))
    n_params = len(in_names)
    n_outs = len(out_avals)
    all_in_names = list(in_names) + list(out_names)
    if partition_name is not None:
        all_in_names.append(partition_name)

    donate = tuple(range(n_params, n_params + n_outs))

    def _body(*args):
        operands = list(args)
        if partition_name is not None:
            operands.append(partition_id_tensor())
        outs = _bass_exec_p.bind(
            *operands,
            out_avals=tuple(out_avals),
            in_names=tuple(all_in_names),
            out_names=tuple(out_names),
            lowering_input_output_aliases=(),
            sim_require_finite=True,
            sim_require_nnan=True,
            nc=nc,
        )
        return tuple(outs)

    devices = jax.devices()[:n_cores]
    mesh = Mesh(np.asarray(devices), ("core",))
    in_specs = (PartitionSpec("core"),) * (n_params + n_outs)
    out_specs = (PartitionSpec("core"),) * len(out_names)
    sharded = jax.jit(
        shard_map(
            _body, mesh=mesh, in_specs=in_specs, out_specs=out_specs, check_rep=False
        ),
        donate_argnums=donate,
        keep_unused=True,
    )

    def run(in_maps):
        per_core = [[np.asarray(m[name]) for name in in_names] for m in in_maps]
        concat_in = [
            np.concatenate([per_core[cc][i] for cc in range(n_cores)], axis=0)
            for i in range(n_params)
        ]
        concat_zeros = [
            np.zeros((n_cores * z.shape[0], *z.shape[1:]), z.dtype) for z in zero_outs
        ]
        out_arrs = sharded(*concat_in, *concat_zeros)
        jax.block_until_ready(out_arrs)
        return [
            {
                name: np.asarray(out_arrs[i]).reshape(n_cores, *out_avals[i].shape)[cc]
                for i, name in enumerate(out_names)
            }
            for cc in range(n_cores)
        ]

    return run


_CACHE = {}


def _get_runner(cfg: Cfg, meta: Meta, repeats: int = 1):
    key = (cfg, meta.key, repeats)
    if key in _CACHE:
        return _CACHE[key]
    nc = bacc.Bacc(
        "TRN2",
        target_bir_lowering=False,
        debug=False,
        num_devices=cfg.NC,
        num_swdge_queues=4,
    )
    build(nc, cfg, meta, repeats)
    nc.compile()
    run = make_runner(nc, cfg.NC)
    _CACHE[key] = run
    return run


def kernel(
    x, edge_index, W1, att_src1, att_dst1, b1, W2, att_src2, att_dst2, b2, _cfg=None
):
    cfg = _cfg or Cfg()
    in_maps, meta = preprocess(x, edge_index, cfg)
    shared = {
        "W1": np.asarray(W1, np.float32),
        "att_src1": np.asarray(att_src1, np.float32),
        "att_dst1": np.asarray(att_dst1, np.float32),
        "b1": np.asarray(b1, np.float32),
        "W2": np.asarray(W2, np.float32),
        "att_src2": np.asarray(att_src2, np.float32),
        "att_dst2": np.asarray(att_dst2, np.float32),
        "b2": np.asarray(b2, np.float32),
    }
    for m in in_maps:
        m.update(shared)
    run = _get_runner(cfg, meta)
    res = run(in_maps)
    out = np.concatenate([r["out"][: cfg.NPC] for r in res], axis=0)
    return out.astype(np.float32)

